# revision 6
# baseline (speedup 1.0000x reference)
"""Trainium2 Bass kernel for PcConvBp (predictive-coding conv block).

Math (per reference): y = relu(conv3x3_same(x, w_ff)); yp = pad(y,1);
5 iters of yp += (LR/||r||)*C^T(r) with r = x - conv_valid(yp, w_fb);
out = yp[:,:,1:-1,1:-1] + conv1x1(x, w_bypass).

Kernel uses the equivalent r-space recurrence (validated to 1e-16):
  u = y + byp; r0 = x - C(pad(y,1)); nsq = sum(r^2)  [global, AllReduced]
  for t in 0..4: a = LR/sqrt(nsq); tfull = C^T(r)  [114x114]
                 u += a * tfull[1:-1,1:-1]
                 if t<4: r -= a*C(tfull); nsq = sum(r^2)

Sharding: data-parallel over batch, 2 images/core on 8 cores; each image's
64 channels live on 64 partitions (2 images -> 128 partitions, block-diag
weights). Only the scalar nsq is AllReduced each iteration.
"""
import sys
sys.path.insert(0, "/opt/trn_rl_repo")
import numpy as np

NCORES = 8
B, C, H, W = 16, 64, 112, 112
NUM_ITERS, LR = 5, 0.01

_cache = {}


def _build(reps=1):
    # reps>1 replicates the compute body back-to-back (garbage values after
    # rep 1) purely so wall-clock deltas isolate HW time from dispatch cost
    import concourse.bacc as bacc
    import concourse.tile as tile
    from concourse import mybir

    F32 = mybir.dt.float32
    F32R = mybir.dt.float32r
    ADD = mybir.AluOpType.add
    SUB = mybir.AluOpType.subtract
    MUL = mybir.AluOpType.mult
    AX = mybir.AxisListType.X
    RELU = mybir.ActivationFunctionType.Relu
    SQRT = mybir.ActivationFunctionType.Sqrt

    nc = bacc.Bacc("TRN2", target_bir_lowering=False, debug=False,
                   num_devices=NCORES)

    X = nc.dram_tensor("X", [128, H, W], F32, kind="ExternalInput").ap()
    WFF = nc.dram_tensor("WFF", [128, 9, 128], F32, kind="ExternalInput").ap()
    WCT = nc.dram_tensor("WCT", [128, 9, 128], F32, kind="ExternalInput").ap()
    WC = nc.dram_tensor("WC", [128, 9, 128], F32, kind="ExternalInput").ap()
    WBYP = nc.dram_tensor("WBYP", [128, 128], F32, kind="ExternalInput").ap()
    OUT = nc.dram_tensor("OUT", [128, H, W], F32, kind="ExternalOutput").ap()

    NBLK = H // 4          # 28 blocks of 4 output rows
    NT = (H + 2 + 3) // 4  # 29 blocks covering the 114-row t canvas

    with tile.TileContext(nc) as tc:
        with (
            tc.tile_pool(name="sb", bufs=1) as sb,
            tc.tile_pool(name="psA", bufs=3, space="PSUM") as psA,
            tc.tile_pool(name="psB", bufs=2, space="PSUM") as psB,
            tc.tile_pool(name="psS", bufs=1, space="PSUM") as psS,
            tc.tile_pool(name="psb2", bufs=1, space="PSUM") as psb2,
            tc.tile_pool(name="dram", bufs=10, space="DRAM") as dpool,
        ):
            canv = sb.tile([128, 116, 116], F32R)   # x, then r (ring of 2)
            canv2 = sb.tile([128, 114, 114], F32R)  # pad(y,1), then tfull
            u = sb.tile([128, H, W], F32)           # x stage, then output acc
            wff = sb.tile([128, 9, 128], F32R)
            wct = sb.tile([128, 9, 128], F32R)
            wc = sb.tile([128, 9, 128], F32R)
            wbyp = sb.tile([128, 128], F32R)
            ssq_part = sb.tile([128, NBLK], F32)
            sq_scr = sb.tile([128, 448], F32)
            ssq_red = sb.tile([128, 1], F32)
            ones_col = sb.tile([128, 1], F32)
            ones_row = sb.tile([1, 128], F32)
            neg_row = sb.tile([1, 128], F32)
            sone = sb.tile([128, 1], F32)
            a_bc = sb.tile([128, 1], F32)
            na_bc = sb.tile([128, 1], F32)
            sc = sb.tile([1, 1], F32)
            gsum = sb.tile([1, 1], F32)
            rc = sb.tile([1, 1], F32)
            at = sb.tile([1, 1], F32)

            nc.vector.memset(ones_col[:], 1.0)
            nc.vector.memset(ones_row[:], 1.0)
            nc.vector.memset(neg_row[:], -1.0)
            nc.vector.memset(sone[:], 1.0)

            # memset can't target f32r tiles: zero the canvas pad rings by
            # ACT-copying from a zeroed f32 scratch
            zsrc = sb.tile([128, 232], F32)
            nc.vector.memset(zsrc[:], 0.0)
            nc.scalar.copy(canv[:, 0:2, :], zsrc[:, 0:232])
            nc.scalar.copy(canv[:, 114:116, :], zsrc[:, 0:232])
            nc.scalar.copy(canv[:, 2:114, 0:2], zsrc[:, 0:224])
            nc.scalar.copy(canv[:, 2:114, 114:116], zsrc[:, 0:224])
            nc.scalar.copy(canv2[:, 0:1, :], zsrc[:, 0:114])
            nc.scalar.copy(canv2[:, 113:114, :], zsrc[:, 0:114])
            nc.scalar.copy(canv2[:, 1:113, 0:1], zsrc[:, 0:112])
            nc.scalar.copy(canv2[:, 1:113, 113:114], zsrc[:, 0:112])

            nc.gpsimd.dma_start(wff[:], WFF[:])
            nc.gpsimd.dma_start(wct[:], WCT[:])
            nc.gpsimd.dma_start(wc[:], WC[:])
            nc.gpsimd.dma_start(wbyp[:], WBYP[:])

            # stage x into u (f32), then ACT-copy (casts) into canv interior
            for q in range(4):
                nc.sync.dma_start(u[:, 28 * q:28 * (q + 1), :],
                                  X[:, 28 * q:28 * (q + 1), :])
            for b in range(NBLK):
                nc.scalar.copy(canv[:, 2 + 4 * b:6 + 4 * b, 2:114],
                               u[:, 4 * b:4 * b + 4, :])

            def _body(write_out):
                # ---- Phase A-1: y = relu(ff conv), u = y + byp ----
                for b in range(NBLK):
                    p = psA.tile([128, 448], F32)
                    for k in range(9):
                        m, n = divmod(k, 3)
                        nc.tensor.matmul(
                            p[:], lhsT=wff[:, k, :],
                            rhs=canv[:, 1 + 4 * b + m:5 + 4 * b + m,
                                     1 + n:113 + n],
                            start=(k == 0), stop=(k == 8))
                    pb = psB.tile([128, 448], F32)
                    nc.tensor.matmul(pb[:], lhsT=wbyp[:],
                                     rhs=canv[:, 2 + 4 * b:6 + 4 * b, 2:114],
                                     start=True, stop=True)
                    nc.scalar.activation(canv2[:, 1 + 4 * b:5 + 4 * b, 1:113],
                                         p[:], RELU)
                    nc.vector.tensor_tensor(
                        u[:, 4 * b:4 * b + 4, :],
                        in0=canv2[:, 1 + 4 * b:5 + 4 * b, 1:113],
                        in1=pb[:], op=ADD)

                # ---- Phase B-1: r = x - C(pad(y,1)), ssq partials ----
                for b in range(NBLK):
                    p = psA.tile([128, 448], F32)
                    for k in range(9):
                        m, n = divmod(k, 3)
                        nc.tensor.matmul(
                            p[:], lhsT=wc[:, k, :],
                            rhs=canv2[:, 4 * b + m:4 * b + m + 4, n:n + 112],
                            start=(k == 0), stop=(k == 8))
                    win = canv[:, 2 + 4 * b:6 + 4 * b, 2:114]
                    nc.vector.tensor_tensor(win, in0=win, in1=p[:], op=SUB)
                    nc.vector.scalar_tensor_tensor(
                        sq_scr[:], in0=win, scalar=sone[:], in1=win,
                        op0=MUL, op1=MUL, accum_out=ssq_part[:, b:b + 1])

                for t in range(NUM_ITERS):
                    # scalar chain part 1: reduce ssq, launch AllReduce
                    nc.vector.tensor_reduce(ssq_red[:], ssq_part[:], axis=AX,
                                            op=ADD)
                    pc = psS.tile([1, 1], F32)
                    nc.tensor.matmul(pc[:], lhsT=ones_col[:], rhs=ssq_red[:],
                                     start=True, stop=True)
                    nc.scalar.copy(sc[:], pc[:])
                    cin = dpool.tile([1, 1], F32)
                    cout = dpool.tile([1, 1], F32)
                    nc.sync.dma_start(cin[:], sc[:])
                    nc.gpsimd.collective_compute(
                        "AllReduce", ADD,
                        replica_groups=[list(range(NCORES))],
                        ins=[cin.opt()], outs=[cout.opt()])
                    nc.sync.dma_start(gsum[:], cout[:])

                    # ---- Phase A_t: tfull = C^T(r) -> canv2 (overlaps
                    # AllReduce)
                    for b in range(NT):
                        rows = 4 if b < NT - 1 else 2
                        nn_ = rows * 114
                        p = psA.tile([128, nn_], F32)
                        for k in range(9):
                            m, n = divmod(k, 3)
                            r0 = 4 * b + 2 - m
                            nc.tensor.matmul(
                                p[:], lhsT=wct[:, k, :],
                                rhs=canv[:, r0:r0 + rows, 2 - n:116 - n],
                                start=(k == 0), stop=(k == 8))
                        nc.scalar.copy(canv2[:, 4 * b:4 * b + rows, :], p[:])

                    # scalar chain part 2: a = LR/sqrt(nsq), broadcast +a/-a
                    nc.vector.reciprocal(rc[:], gsum[:])
                    nc.scalar.activation(at[:], rc[:], SQRT, scale=LR * LR)
                    p1 = psb2.tile([128, 1], F32)
                    nc.tensor.matmul(p1[:], lhsT=ones_row[:], rhs=at[:],
                                     start=True, stop=True)
                    nc.scalar.copy(a_bc[:], p1[:])
                    p2 = psb2.tile([128, 1], F32)
                    nc.tensor.matmul(p2[:], lhsT=neg_row[:], rhs=at[:],
                                     start=True, stop=True)
                    nc.scalar.copy(na_bc[:], p2[:])

                    # u += a * tfull[1:-1, 1:-1]
                    for b in range(NBLK):
                        uw = u[:, 4 * b:4 * b + 4, :]
                        nc.vector.scalar_tensor_tensor(
                            uw, in0=canv2[:, 1 + 4 * b:5 + 4 * b, 1:113],
                            scalar=a_bc[:], in1=uw, op0=MUL, op1=ADD)
                        if write_out and t == NUM_ITERS - 1:
                            nc.sync.dma_start(OUT[:, 4 * b:4 * b + 4, :], uw)

                    # ---- Phase B_t: r -= a*C(tfull), ssq partials ----
                    if t < NUM_ITERS - 1:
                        for b in range(NBLK):
                            p = psA.tile([128, 448], F32)
                            for k in range(9):
                                m, n = divmod(k, 3)
                                nc.tensor.matmul(
                                    p[:], lhsT=wc[:, k, :],
                                    rhs=canv2[:, 4 * b + m:4 * b + m + 4,
                                              n:n + 112],
                                    start=(k == 0), stop=(k == 8))
                            win = canv[:, 2 + 4 * b:6 + 4 * b, 2:114]
                            nc.vector.scalar_tensor_tensor(
                                win, in0=p[:], scalar=na_bc[:], in1=win,
                                op0=MUL, op1=ADD)
                            nc.vector.scalar_tensor_tensor(
                                sq_scr[:], in0=win, scalar=sone[:], in1=win,
                                op0=MUL, op1=MUL,
                                accum_out=ssq_part[:, b:b + 1])

            for _rep in range(reps):
                _body(_rep == reps - 1)

    nc.finalize()
    return nc


def _get_nc():
    if "nc" not in _cache:
        _cache["nc"] = _build()
    return _cache["nc"]


def _pack_weights(w_ff, w_fb, w_bypass):
    wff_h = np.zeros((128, 9, 128), np.float32)
    wct_h = np.zeros((128, 9, 128), np.float32)
    wc_h = np.zeros((128, 9, 128), np.float32)
    wb_h = np.zeros((128, 128), np.float32)
    for k in range(9):
        m, n = divmod(k, 3)
        ff = np.asarray(w_ff[:, :, m, n], np.float32).T   # [ci, co]
        ct = np.asarray(w_fb[:, :, m, n], np.float32)     # [i, o] for C^T
        cc = np.asarray(w_fb[:, :, m, n], np.float32).T   # [i, o] for C
        for blk in range(2):
            s = 64 * blk
            wff_h[s:s + 64, k, s:s + 64] = ff
            wct_h[s:s + 64, k, s:s + 64] = ct
            wc_h[s:s + 64, k, s:s + 64] = cc
    byp = np.asarray(w_bypass[:, :, 0, 0], np.float32).T
    for blk in range(2):
        s = 64 * blk
        wb_h[s:s + 64, s:s + 64] = byp
    return wff_h, wct_h, wc_h, wb_h


def kernel(x, w_ff, w_fb, w_bypass, layer_idx=None, **_unused):
    from concourse.bass_utils import run_bass_kernel_spmd

    x = np.ascontiguousarray(np.asarray(x, np.float32))
    wff_h, wct_h, wc_h, wb_h = _pack_weights(w_ff, w_fb, w_bypass)

    nc = _get_nc()
    in_maps = []
    for i in range(NCORES):
        xi = x[2 * i:2 * i + 2].reshape(128, H, W)
        in_maps.append({"X": np.ascontiguousarray(xi), "WFF": wff_h,
                        "WCT": wct_h, "WC": wc_h, "WBYP": wb_h})
    res = run_bass_kernel_spmd(nc, in_maps, list(range(NCORES)))
    out = np.empty((B, C, H, W), np.float32)
    for i in range(NCORES):
        out[2 * i:2 * i + 2] = res.results[i]["OUT"].reshape(2, C, H, W)
    return out
